# revision 31
# baseline (speedup 1.0000x reference)
"""Trainium2 Bass kernel: autoregressive LSTM decoder (nn_DecoderAR).

Reference computation (per step t over HORIZON=24):
    inp   = concat([x_t, y_prev])                      (B, 8)
    gates = inp @ W_ih.T + b_ih + h @ W_hh.T + b_hh    (B, 2048)
    i, f, g, o = split(gates); sigmoid/tanh
    c = f*c + i*g ; h = o*tanh(c)
    logit = h @ fc_w.T + fc_b ; y_prev = sigmoid(logit)
Output: logits (B, 24, 1).

Sharding: data-parallel over batch (8192 -> 8 cores x 1024), weights
replicated.  Everything on-chip is kept transposed (batch on the matmul
free dim, gate/hid dims on partitions) so the recurrence needs no
transposes: the gate matmuls are W~ @ [h; x_t; y] with W~ as the
stationary operand, h as the streaming operand, and h_new is produced
directly in streaming layout for the next step.

Per core, per step, per 512-row batch half:
  - 16 gate chains (one per 128-row gate chunk), each 5 matmuls:
    4x (K=128) W_hh chunks + 1x (K=8) [x_t; y] chunk, accumulated in PSUM.
  - biases fused into the ScalarE sigmoid/tanh that drains PSUM.
  - c/h elementwise on VectorE (c in fp32, h written as bf16 = next rhs).
  - fc logit = 4 accumulating (K=128 -> M=1) matmuls; y = sigmoid(logit)
    written into the next step's [x;y] rhs tile by one ScalarE op.
The two batch halves are interleaved so one half's ACT/DVE tail hides
under the other half's matmuls; fc/y of a half-step are emitted after the
other half's chains for the same reason.
"""

import numpy as np
import ml_dtypes

import concourse.bass as bass
import concourse.mybir as mybir
import concourse.tile as tile
from concourse import bacc
from concourse.bass_utils import run_bass_kernel_spmd

B, HORIZON, NCOV, HID = 8192, 24, 7, 512
NCORES = 8
BL = B // NCORES          # batch rows per core (1024)
P = 128
KC = HID // P             # hid chunks (4)
NMC = 4 * HID // P        # gate chunks (16)
KE = NCOV + 1             # extra contraction rows: 7 covariates + y

F32 = mybir.dt.float32
BF16 = mybir.dt.bfloat16
AF = mybir.ActivationFunctionType
BF16NP = ml_dtypes.bfloat16

# gate order in the PyTorch LSTMCell layout: i, f, g, o
GATE_FUNC = [AF.Sigmoid, AF.Sigmoid, AF.Tanh, AF.Sigmoid]

# Pack the 16 K=8 [x;y] matmuls per half-step into groups of 4 concurrent
# matmuls using 32-row PE array tiling (tile_position=(32g, 0)); the x/y
# rows are replicated into four 32-partition strips so each strip feeds its
# own row-group of the array.
PACK_EXTRAS = False


def build_program(horizon=HORIZON, bl=BL, repeats=1, pack=None):
    """Build the single-core Bass program (identical on all cores).

    repeats>1 re-runs the whole recurrence (benchmarking only: the extra
    passes reuse the same x slots / output rows, so results are those of
    the last pass, which no longer matches the reference)."""
    if pack is None:
        pack = PACK_EXTRAS
    bh = bl // 2  # batch half = matmul free dim (512)
    ke = P if pack else KE  # x/y rows (replicated into 4 strips if packed)
    # Bacc (not plain Bass): its finalize() pipeline legalizes multi-wait
    # instructions into event-semaphore ops; raw Bass output makes walrus
    # fail with "Too many sync wait commands".
    nc = bacc.Bacc(None)

    xrep = nc.declare_dram_parameter("xrep", [ke, horizon, bl], BF16, False)
    h0 = nc.declare_dram_parameter("h0", [P, KC, bl], BF16, False)
    c0 = nc.declare_dram_parameter("c0", [P, KC, bl], F32, False)
    whh = nc.declare_dram_parameter("whh", [P, KC, 4 * HID], BF16, False)
    we = nc.declare_dram_parameter("we", [ke, NMC, P], BF16, False)
    bias = nc.declare_dram_parameter("bias", [P, NMC], F32, False)
    fcw = nc.declare_dram_parameter("fcw", [P, KC], BF16, False)
    fcb = nc.declare_dram_parameter("fcb", [1, 1], BF16, False)
    out = nc.declare_dram_parameter("out", [horizon, bl], F32, True)

    with tile.TileContext(nc) as tc:
        with (
            tc.tile_pool(name="singles", bufs=1) as singles,
            tc.tile_pool(name="gacts", bufs=8) as gacts,
            tc.tile_pool(name="tails", bufs=6) as tails,
            tc.tile_pool(name="ps_g", bufs=5, space="PSUM") as ps_g,
            tc.tile_pool(name="ps_fc", bufs=3, space="PSUM") as ps_fc,
        ):
            # --- resident tensors, loaded once ---
            xs_sb = []
            for hf in range(2):
                xt = singles.tile([ke, horizon, bh], BF16, tag=f"xrep{hf}", name=f"xrep{hf}")
                nc.sync.dma_start(xt[:], xrep[:, :, hf * bh:(hf + 1) * bh])
                xs_sb.append(xt)
            whh_sb = singles.tile([P, KC, 4 * HID], BF16, tag="whh", name="whh")
            nc.sync.dma_start(whh_sb[:], whh[:])
            we_sb = singles.tile([ke, NMC, P], BF16, tag="we", name="we")
            nc.sync.dma_start(we_sb[:], we[:])
            bias_sb = singles.tile([P, NMC], F32, tag="bias", name="bias")
            nc.sync.dma_start(bias_sb[:], bias[:])
            fcw_sb = singles.tile([P, KC], BF16, tag="fcw", name="fcw")
            nc.sync.dma_start(fcw_sb[:], fcw[:])
            fcb_sb = singles.tile([1, 1], BF16, tag="fcb", name="fcb")
            nc.sync.dma_start(fcb_sb[:], fcb[:])
            ones_sb = singles.tile([1, bh], BF16, tag="ones", name="ones")
            nc.vector.memset(ones_sb[:], 1.0)

            # ping-pong h (bf16, = matmul rhs) and c (fp32), per half
            h_sb = [[singles.tile([P, KC, bh], BF16, tag=f"h{hf}_{i}", name=f"h{hf}_{i}")
                     for i in range(2)] for hf in range(2)]
            c_sb = [[singles.tile([P, KC, bh], F32, tag=f"c{hf}_{i}", name=f"c{hf}_{i}")
                     for i in range(2)] for hf in range(2)]
            for hf in range(2):
                csl = slice(hf * bh, (hf + 1) * bh)
                nc.sync.dma_start(h_sb[hf][0][:], h0[:, :, csl])
                nc.sync.dma_start(c_sb[hf][0][:], c0[:, :, csl])

            def emit_chains(hf, t, tg):
                """Gate chains + c/h updates for one (half, step)."""
                cur, nxt = tg % 2, (tg + 1) % 2
                xs = xs_sb[hf]
                hcur, hnxt = h_sb[hf][cur], h_sb[hf][nxt]
                ccur, cnxt = c_sb[hf][cur], c_sb[hf][nxt]
                for j in range(KC):
                    gt = []
                    if pack:
                        # 4 concurrent K=8 [x;y] matmuls, one per 32-row
                        # array strip (tile_position), one per gate; then
                        # the K=128 hh chunks accumulate on top.
                        pss = []
                        for g in range(4):
                            mc = g * KC + j
                            ps = ps_g.tile([P, bh], F32, tag="gps", name="gps")
                            nc.tensor.matmul(
                                ps[:],
                                we_sb[32 * g:32 * g + KE, mc, :],
                                xs[32 * g:32 * g + KE, t, :],
                                start=True, stop=False,
                                tile_position=(32 * g, 0),
                            )
                            pss.append(ps)
                        for g in range(4):
                            mc = g * KC + j
                            for kk in range(KC):
                                nc.tensor.matmul(
                                    pss[g],
                                    whh_sb[:, kk, mc * P:(mc + 1) * P],
                                    hcur[:, kk, :],
                                    start=False, stop=(kk == KC - 1),
                                )
                            a = gacts.tile([P, bh], F32, tag="gact", name="gact")
                            nc.scalar.activation(
                                a[:], pss[g][:], GATE_FUNC[g],
                                bias=bias_sb[:, mc:mc + 1],
                            )
                            gt.append(a)
                    else:
                        for g in range(4):
                            mc = g * KC + j
                            ps = ps_g.tile([P, bh], F32, tag="gps", name="gps")
                            # 4x K=128 hh chunks first, K=8 [x;y] chunk last
                            # (y of this step is produced late; by chain 1 it
                            # is long done, and even chain 0 only needs it
                            # for its 5th matmul).
                            for kk in range(KC):
                                nc.tensor.matmul(
                                    ps[:],
                                    whh_sb[:, kk, mc * P:(mc + 1) * P],
                                    hcur[:, kk, :],
                                    start=(kk == 0), stop=False,
                                )
                            nc.tensor.matmul(
                                ps[:], we_sb[:, mc, :], xs[:, t, :],
                                start=False, stop=True,
                            )
                            a = gacts.tile([P, bh], F32, tag="gact", name="gact")
                            nc.scalar.activation(
                                a[:], ps[:], GATE_FUNC[g],
                                bias=bias_sb[:, mc:mc + 1],
                            )
                            gt.append(a)
                    it, ft, gg, ot = gt
                    t1 = tails.tile([P, bh], F32, tag="t1", name="t1")
                    t2 = tails.tile([P, bh], F32, tag="t2", name="t2")
                    nc.vector.tensor_mul(t1[:], it[:], gg[:])
                    nc.vector.tensor_mul(t2[:], ft[:], ccur[:, j, :])
                    nc.vector.tensor_add(cnxt[:, j, :], t1[:], t2[:])
                    tnh = tails.tile([P, bh], F32, tag="tnh", name="tnh")
                    nc.scalar.activation(tnh[:], cnxt[:, j, :], AF.Tanh)
                    nc.vector.tensor_mul(hnxt[:, j, :], ot[:], tnh[:])

            def emit_tail(hf, t, tg):
                """fc logit (+fc_b) + y recirculation for one (half, step)."""
                nxt = (tg + 1) % 2
                hnxt = h_sb[hf][nxt]
                fc_ps = ps_fc.tile([1, bh], F32, tag="fc", name="fc_ps")
                for j in range(KC):
                    nc.tensor.matmul(
                        fc_ps[:], fcw_sb[:, j:j + 1], hnxt[:, j, :],
                        start=(j == 0), stop=False,
                    )
                # fc_b folded in as a rank-1 matmul against a ones row, so
                # the PSUM tile holds the finished logit (DMA'd out as-is).
                nc.tensor.matmul(
                    fc_ps[:], fcb_sb[:], ones_sb[:],
                    start=False, stop=True,
                )
                if tg + 1 < horizon * repeats:
                    # y slot is row 0 of each strip (engine writes need
                    # start partition 0/32/64/96)
                    if pack:
                        for g in range(4):
                            nc.scalar.activation(
                                xs_sb[hf][32 * g:32 * g + 1,
                                          (t + 1) % horizon, :],
                                fc_ps[:], AF.Sigmoid,
                            )
                    else:
                        nc.scalar.activation(
                            xs_sb[hf][0:1, (t + 1) % horizon, :], fc_ps[:],
                            AF.Sigmoid,
                        )
                # bounce logit through SBUF (DMA cannot read PSUM); engine
                # writes must start at partition 0, hence a partition-0 strip
                osl = tails.tile([1, bh], F32, tag="osl", name="osl")
                nc.scalar.copy(osl[:], fc_ps[:])
                nc.sync.dma_start(out[t:t + 1, hf * bh:(hf + 1) * bh], osl[:])

            pending = None
            for rep in range(repeats):
                for t in range(horizon):
                    tg = rep * horizon + t
                    for hf in range(2):
                        emit_chains(hf, t, tg)
                        if pending is not None:
                            emit_tail(*pending)
                        pending = (hf, t, tg)
            emit_tail(*pending)

    nc.finalize()
    return nc


def prepare_inputs(future_x, h_enc, c_enc, y0, W_ih, W_hh, b_ih, b_hh,
                   fc_w, fc_b, horizon=HORIZON, bl=BL, ncores=NCORES,
                   pack=None):
    """Host-side shard + layout prep. Returns list of per-core input dicts."""
    if pack is None:
        pack = PACK_EXTRAS
    future_x = np.asarray(future_x, np.float32)
    h_enc = np.asarray(h_enc, np.float32)
    c_enc = np.asarray(c_enc, np.float32)
    y0 = np.asarray(y0, np.float32)
    W_ih = np.asarray(W_ih, np.float32)
    W_hh = np.asarray(W_hh, np.float32)
    b_ih = np.asarray(b_ih, np.float32)
    b_hh = np.asarray(b_hh, np.float32)
    fc_w = np.asarray(fc_w, np.float32)
    fc_b = np.asarray(fc_b, np.float32)

    # replicated weights
    # whh[p, k, m] = W_hh[m, k*128+p]
    whh_host = np.ascontiguousarray(
        W_hh.T.reshape(KC, P, 4 * HID).transpose(1, 0, 2)).astype(BF16NP)
    # we[r, mc, c] = W_ih[mc*128+c, col(r)]; row 0 = y weights, rows 1-7 = x
    # covariates (y lives at partition 0 so the per-step ScalarE write of
    # y = sigmoid(logit) targets start partition 0).
    we_host = np.ascontiguousarray(
        W_ih[:, [NCOV] + list(range(NCOV))].reshape(NMC, P, KE)
        .transpose(2, 0, 1)).astype(BF16NP)
    if pack:
        # replicate into 4 strips of 32 partitions (one per PE row-group)
        we_p = np.zeros((P, NMC, P), BF16NP)
        for g in range(4):
            we_p[32 * g:32 * g + KE] = we_host
        we_host = we_p
    bias_host = np.ascontiguousarray((b_ih + b_hh).reshape(NMC, P).T)
    fcw_host = np.ascontiguousarray(fc_w.reshape(KC, P).T).astype(BF16NP)
    fcb_host = np.full((1, 1), float(fc_b[0]), BF16NP)

    in_maps = []
    for core in range(ncores):
        sl = slice(core * bl, (core + 1) * bl)
        ke = P if pack else KE
        xrep_host = np.zeros((ke, horizon, bl), BF16NP)
        xt = future_x[sl, :horizon].transpose(2, 1, 0).astype(BF16NP)
        y0t = y0[sl, 0].astype(BF16NP)
        for g in range(4 if pack else 1):
            # row 0 of each strip = y slot, rows 1-7 = x_t covariates
            xrep_host[32 * g + 1:32 * g + KE] = xt
            # y slot for step 0 = y0 (later steps filled on device)
            xrep_host[32 * g, 0, :] = y0t
        h0_host = np.ascontiguousarray(
            h_enc[sl].T.reshape(KC, P, bl).transpose(1, 0, 2)).astype(BF16NP)
        c0_host = np.ascontiguousarray(
            c_enc[sl].T.reshape(KC, P, bl).transpose(1, 0, 2)).astype(np.float32)
        in_maps.append({
            "xrep": xrep_host,
            "h0": h0_host,
            "c0": c0_host,
            "whh": whh_host,
            "we": we_host,
            "bias": bias_host.astype(np.float32),
            "fcw": fcw_host,
            "fcb": fcb_host,
        })
    return in_maps


def run(inputs, trace=False, **kwargs):
    """Run on 8 NeuronCores; returns (full_output, BassKernelResults)."""
    nc = build_program()
    in_maps = prepare_inputs(**inputs)
    res = run_bass_kernel_spmd(nc, in_maps, core_ids=list(range(NCORES)),
                               trace=trace, **kwargs)
    full = np.empty((B, HORIZON, 1), np.float32)
    for core in range(NCORES):
        o = np.asarray(res.results[core]["out"], np.float32)  # (HORIZON, BL)
        full[core * BL:(core + 1) * BL, :, 0] = o.T
    return full, res


def kernel(**inputs):
    out, _ = run(inputs)
    return out
